# revision 22
# baseline (speedup 1.0000x reference)
"""GQA attention kernel for Trainium2, tensor-parallel over heads across 8 cores.

Problem: T=2048, D=4096, H=32 q-heads, G=8 kv-heads, HD=128.
Per core: 4 q heads + 1 kv head (group), full T.
All matmul operands bf16 (FWL weight loads, half the HBM traffic); psum f32.

Fully interleaved schedule: after the projections/rope/transpose of row
blocks 4g..4g+3, the attention supertile g (scores/exp/attn-V) and its
out-proj rows run immediately, so attention matmuls fill rope-wait bubbles
and output DMA overlaps the next supertile's x streaming.

  q = rmsnorm(x @ Wq_c) -> rope -> qT [HD, h, T] kept in SBUF
  k = rmsnorm(x @ Wk_c) -> rope -> kT [HD, T]; v natural [T, HD]
  scoresT[j,i] blocks = kT_j.T @ qT (causal-blocked)
  expT = exp(scoresT/sqrt(HD)) * diag mask
  denom: DVE accumulates den[128,512] += expT_j (bf16), then one
         ones[128,128]^T @ den matmul -> denom on every psum partition
  attnT[HD, T] += v_j.T @ expT_j; normalized by reciprocal(denom)
  out_partial = attnT.T @ Wo_c (bf16, row-sharded) -> host sums the 8 partials.
"""
import sys

sys.path.insert(0, '/opt/trn_rl_repo')

import numpy as np
import ml_dtypes

import concourse.bass as bass
import concourse.bacc as bacc
import concourse.mybir as mybir
import concourse.tile as tile
from concourse.bass_utils import run_bass_kernel_spmd

F32 = mybir.dt.float32
F32R = mybir.dt.float32r
BF16 = mybir.dt.bfloat16
AF = mybir.ActivationFunctionType
OP = mybir.AluOpType

T = 2048
D = 4096
H = 32
G = 8
HD = 128
NCORES = 8
HPC = H // NCORES          # 4 q heads per core
NB = T // 128              # 16 row/col blocks
NSUP = NB // 4             # 4 supertiles of 512 queries
DKT = D // 128             # 32 contraction tiles for projections
NOC = D // 512             # 8 out-proj column blocks
EPS = 1e-6
ISQ = 1.0 / float(np.sqrt(HD))


def _rotview(ap):
    """[128, 128] AP -> [128, 2, 64] view reading cols 64:128 then 0:64."""
    return bass.AP(ap.tensor, ap.offset + 64, [list(ap.ap[0]), [-64, 2], [1, 64]])


def _emit(nc, tc):
    xt = nc.dram_tensor("xt", [NB, 128, DKT * 128], BF16, kind="ExternalInput")
    wq = nc.dram_tensor("wq", [128, DKT * 512], BF16, kind="ExternalInput")
    wkv = nc.dram_tensor("wkv", [128, DKT * 256], BF16, kind="ExternalInput")
    wo = nc.dram_tensor("wo", [128, HPC * NOC * 512], BF16, kind="ExternalInput")
    tbl = nc.dram_tensor("tbl", [4, T, HD], F32, kind="ExternalInput")
    tri01 = nc.dram_tensor("tri01", [128, 128], BF16, kind="ExternalInput")
    ones128 = nc.dram_tensor("ones128", [128, 128], BF16, kind="ExternalInput")
    ident = nc.dram_tensor("ident", [128, 128], F32R, kind="ExternalInput")
    out = nc.dram_tensor("out", [T, D], BF16, kind="ExternalOutput")

    import contextlib
    ctx = contextlib.ExitStack()
    with ctx:
        const_p = ctx.enter_context(tc.tile_pool(name="const", bufs=1))
        tri_sb = const_p.tile([128, 128], BF16)
        oc_sb = const_p.tile([128, 128], BF16)
        id_sb = const_p.tile([128, 128], F32R)
        epsb = const_p.tile([128, 1], F32)

        # persistent SBUF: kT, v, qT, attnT, Wo (all bf16)
        pers = ctx.enter_context(tc.tile_pool(name="pers", bufs=1))
        kT = pers.tile([128, T], BF16)
        v_sb = pers.tile([128, NB, 128], BF16)
        qT_sb = pers.tile([128, HPC, T], BF16)
        attnT = pers.tile([128, HPC, T], BF16)
        wo_sb = pers.tile([128, HPC * NOC * 512], BF16)
        wo4 = wo_sb[:].rearrange("p (h n c) -> p h n c", h=HPC, n=NOC)

        wq_p = ctx.enter_context(tc.tile_pool(name="wq_p", bufs=1))
        wkv_p = ctx.enter_context(tc.tile_pool(name="wkv_p", bufs=1))
        xb_p = ctx.enter_context(tc.tile_pool(name="xb_p", bufs=3))
        tbl_p = ctx.enter_context(tc.tile_pool(name="tbl_p", bufs=2))
        scr_p = ctx.enter_context(tc.tile_pool(name="scr_p", bufs=2))
        ro_p = ctx.enter_context(tc.tile_pool(name="ro_p", bufs=6))
        exp_p = ctx.enter_context(tc.tile_pool(name="exp_p", bufs=3, side="right"))
        den_p = ctx.enter_context(tc.tile_pool(name="den_p", bufs=2))
        rc_p = ctx.enter_context(tc.tile_pool(name="rc_p", bufs=2))
        ost_p = ctx.enter_context(tc.tile_pool(name="ost_p", bufs=2))

        # PSUM: mm (psq + po + psd shared) 2 + pskv 1
        #       + big (transpose/scores) 3 + pat 2  = 8 banks
        mm_p = ctx.enter_context(tc.tile_pool(name="mm_p", bufs=2, space="PSUM"))
        pden_p = mm_p
        pskv_p = ctx.enter_context(
            tc.tile_pool(name="pskv_p", bufs=1, space="PSUM"))
        big_p = ctx.enter_context(
            tc.tile_pool(name="big_p", bufs=3, space="PSUM", side="right"))
        pat_p = ctx.enter_context(
            tc.tile_pool(name="pat_p", bufs=2, space="PSUM", side="right"))

        wq_sb = wq_p.tile([128, DKT * 512], BF16)
        wkv_sb = wkv_p.tile([128, DKT * 256], BF16)
        wq3 = wq_sb[:].rearrange("p (k n) -> p k n", k=DKT)
        wkv3 = wkv_sb[:].rearrange("p (k n) -> p k n", k=DKT)

        pending = [None]   # deferred normalize: (attnT_slice, pat, bc)

        def emit_normalize():
            if pending[0] is not None:
                sl, ppat, pbc = pending[0]
                nc.vector.tensor_tensor(sl, ppat, pbc, op=OP.mult)
                pending[0] = None

        def phase1_block(i):
            xb = xb_p.tile([128, DKT * 128], BF16)
            if i == 0:
                # critical first chunks: xb ktile 0 + wq ktile 0
                nc.sync.dma_start(xb[:, 0:256], xt.ap()[0][:, 0:256])
                nc.sync.dma_start(wq_sb[:, 0:512], wq.ap()[:, 0:512])
                for chh in range(1, 16):
                    w = DKT * 128 // 16
                    nc.sync.dma_start(xb[:, chh * w:(chh + 1) * w],
                                      xt.ap()[0][:, chh * w:(chh + 1) * w])
            else:
                for chh in range(2):
                    w = DKT * 128 // 2
                    nc.sync.dma_start(xb[:, chh * w:(chh + 1) * w],
                                      xt.ap()[i][:, chh * w:(chh + 1) * w])
            xb3 = xb[:].rearrange("p (k n) -> p k n", k=DKT)

            tb = tbl_p.tile([128, 4, HD], F32, tag="tb")
            nc.sync.dma_start(
                tb[:],
                tbl.ap()[:, i * 128:(i + 1) * 128, :].transpose([1, 0, 2]))
            cq, sq, ck, sk = (tb[:, 0, :], tb[:, 1, :], tb[:, 2, :],
                              tb[:, 3, :])
            if i == 0:
                for chh in range(1, 32):
                    nc.sync.dma_start(wq_sb[:, chh * 512:(chh + 1) * 512],
                                      wq.ap()[:, chh * 512:(chh + 1) * 512])
                for chh in range(4):
                    w = DKT * 256 // 4
                    nc.sync.dma_start(wkv_sb[:, chh * w:(chh + 1) * w],
                                      wkv.ap()[:, chh * w:(chh + 1) * w])
                nc.sync.dma_start(id_sb[:], ident.ap())
                nc.sync.dma_start(tri_sb[:], tri01.ap())
                nc.sync.dma_start(oc_sb[:], ones128.ap())
                nc.vector.memset(epsb[:], float(HD) * EPS)
            if i == 2:
                # preload Wo (needed from outproj(0), after attention(1) starts)
                for chh in range(8):
                    w = HPC * NOC * 512 // 8
                    nc.sync.dma_start(wo_sb[:, chh * w:(chh + 1) * w],
                                      wo.ap()[:, chh * w:(chh + 1) * w])

            psq = mm_p.tile([128, 512], F32, tag="mm", name="psq")
            # (psq shares the mm tag with po/psd; rotation keeps 2 banks)
            pskv = pskv_p.tile([128, 256], F32)
            for kk in range(DKT):
                nc.tensor.matmul(psq[:], xb3[:, kk, :], wq3[:, kk, :],
                                 start=(kk == 0), stop=(kk == DKT - 1))
            for kk in range(DKT):
                nc.tensor.matmul(pskv[:], xb3[:, kk, :], wkv3[:, kk, :],
                                 start=(kk == 0), stop=(kk == DKT - 1))

            # rms stats for 4 q chunks + 1 k chunk
            ssq = scr_p.tile([128, 8], F32, tag="ssq")
            sqscr = scr_p.tile([128, 128], F32, tag="sqscr")
            for c in range(HPC):
                nc.scalar.activation(sqscr[:], psq[:, c * 128:(c + 1) * 128],
                                     AF.Square, accum_out=ssq[:, c:c + 1])
            nc.scalar.activation(sqscr[:], pskv[:, 0:128],
                                 AF.Square, accum_out=ssq[:, 4:5])
            rstd = scr_p.tile([128, 8], F32, tag="rstd")
            nc.scalar.activation(rstd[:, 0:5], ssq[:, 0:5], AF.Sqrt,
                                 bias=epsb[:])
            nc.vector.reciprocal_approx_fast(rstd[:, 0:5], rstd[:, 0:5])

            # rope (rmsnorm scale fused; sqrt(HD) and qn/kn weights folded
            # into the host-side cos/sin tables)
            roq = ro_p.tile([128, 512], F32R, tag="roq")
            rok = ro_p.tile([128, 128], F32R, tag="rok")
            t1 = scr_p.tile([128, 128], F32, tag="t1")
            t2 = scr_p.tile([128, 128], F32, tag="t2")
            for c in range(HPC):
                ch = psq[:, c * 128:(c + 1) * 128]
                nc.vector.scalar_tensor_tensor(
                    t1[:], ch, rstd[:, c:c + 1], cq,
                    op0=OP.mult, op1=OP.mult)
                nc.vector.scalar_tensor_tensor(
                    t2[:].rearrange("p (a b) -> p a b", a=2), _rotview(ch),
                    rstd[:, c:c + 1], sq.rearrange("p (a b) -> p a b", a=2),
                    op0=OP.mult, op1=OP.mult)
                nc.vector.tensor_add(roq[:, c * 128:(c + 1) * 128], t1[:], t2[:])
            chk = pskv[:, 0:128]
            nc.vector.scalar_tensor_tensor(
                t1[:], chk, rstd[:, 4:5], ck, op0=OP.mult, op1=OP.mult)
            nc.vector.scalar_tensor_tensor(
                t2[:].rearrange("p (a b) -> p a b", a=2), _rotview(chk),
                rstd[:, 4:5], sk.rearrange("p (a b) -> p a b", a=2),
                op0=OP.mult, op1=OP.mult)
            nc.vector.tensor_add(rok[:], t1[:], t2[:])

            # v: psum -> sbuf (f32 -> bf16)
            nc.scalar.copy(v_sb[:, i, :], pskv[:, 128:256])
            return roq, rok

        def phase1_transpose(i, roq, rok):
            # transposes (f32r): 4 q chunks + 1 k via shared psum pool;
            # deferred one block so rope latency is covered by matmuls
            trq = big_p.tile([128, 512], F32, tag="big", name="trq")
            for c in range(HPC):
                nc.tensor.transpose(
                    trq[:, c * 128:(c + 1) * 128].bitcast(F32R),
                    roq[:, c * 128:(c + 1) * 128], id_sb[:])
            trk = big_p.tile([128, 512], F32, tag="big", name="trk")
            nc.tensor.transpose(trk[:, 0:128].bitcast(F32R), rok[:], id_sb[:])

            # qT stays in SBUF (no DRAM spill); copies convert f32 -> bf16
            for c in range(HPC):
                nc.scalar.copy(qT_sb[:, c, i * 128:(i + 1) * 128],
                               trq[:, c * 128:(c + 1) * 128])
            nc.scalar.copy(kT[:, i * 128:(i + 1) * 128], trk[:, 0:128])

        def attention_head(g, h):
                qtile = qT_sb[:, h, g * 512:(g + 1) * 512]
                pat = pat_p.tile([128, 512], F32, tag="pat", name="pat")
                den = den_p.tile([128, 512], BF16, tag="den")
                nj = 4 * g + 4
                for j in range(nj):
                    c = max(0, j - 4 * g)
                    psc = big_p.tile([128, 512], F32, tag="big", name="psc")
                    nc.tensor.matmul(psc[:, c * 128:512],
                                     kT[:, j * 128:(j + 1) * 128],
                                     qtile[:, c * 128:512],
                                     start=True, stop=True)
                    ex = exp_p.tile([128, 512], BF16)
                    nc.scalar.activation(ex[:, c * 128:512], psc[:, c * 128:512],
                                         AF.Exp, scale=ISQ)
                    if j >= 4 * g:
                        nc.vector.tensor_mul(
                            ex[:, c * 128:(c + 1) * 128],
                            ex[:, c * 128:(c + 1) * 128],
                            tri_sb[:])
                    nc.tensor.matmul(pat[:, c * 128:512], v_sb[:, j, :],
                                     ex[:, c * 128:512],
                                     start=(j == 0), stop=(j == nj - 1),
                                     skip_group_check=True)
                    # denominator accumulation on DVE (bf16, 2x mode)
                    if j == 0:
                        nc.vector.tensor_copy(den[:], ex[:, 0:512])
                    else:
                        nc.vector.tensor_add(den[:, c * 128:512],
                                             den[:, c * 128:512],
                                             ex[:, c * 128:512])
                    if j == 1:
                        emit_normalize()
                # single partition-sum matmul per (g,h); ones[128,128]
                # stationary puts the denominator on every psum partition
                psd = pden_p.tile([128, 512], F32, tag="mm", name="psd")
                nc.tensor.matmul(psd[:], oc_sb[:], den[:],
                                 start=True, stop=True)
                bc = rc_p.tile([128, 512], F32, tag="bc")
                nc.vector.reciprocal_approx_fast(bc[:], psd[:])
                pending[0] = (attnT[:, h, g * 512:(g + 1) * 512], pat[:], bc[:])

        def outproj_row(i):
                ot = ost_p.tile([128, D], BF16)
                for n in range(NOC):
                    po = mm_p.tile([128, 512], F32, tag="mm", name="po")
                    for h in range(HPC):
                        nc.tensor.matmul(po[:],
                                         attnT[:, h, i * 128:(i + 1) * 128],
                                         wo4[:, h, n, :],
                                         start=(h == 0), stop=(h == HPC - 1))
                    if n % 2 == 0:
                        nc.scalar.copy(ot[:, n * 512:(n + 1) * 512], po[:])
                    else:
                        nc.vector.tensor_copy(ot[:, n * 512:(n + 1) * 512], po[:])
                nc.sync.dma_start(
                    out.ap()[i * 128:(i + 1) * 128, 0:2048], ot[:, 0:2048])
                nc.sync.dma_start(
                    out.ap()[i * 128:(i + 1) * 128, 2048:4096], ot[:, 2048:4096])

        ros = {}
        for i in range(NB):
            ros[i] = phase1_block(i)
            if 1 <= i <= NB - 4:
                phase1_transpose(i - 1, *ros.pop(i - 1))
        # attention(0) needs only blocks 0-3; the deferred tail transposes
        # (rope long done) fill its exp-wait tensor gaps
        for h in range(HPC):
            attention_head(0, h)
            phase1_transpose(NB - 4 + h, *ros.pop(NB - 4 + h))
        for g in range(1, NSUP):
            for h in range(HPC):
                attention_head(g, h)
                outproj_row(4 * (g - 1) + h)
        emit_normalize()
        for i in range(4 * (NSUP - 1), NB):
            outproj_row(i)


_NC_CACHE = None


def _build():
    global _NC_CACHE
    if _NC_CACHE is None:
        nc = bacc.Bacc("TRN2", target_bir_lowering=False, debug=False)
        with tile.TileContext(nc) as tc:
            _emit(nc, tc)
        nc.compile()
        _NC_CACHE = nc
    return _NC_CACHE


def kernel(x, mask, cos, sin, Wq, Wk, Wv, Wo, qn_w, kn_w):
    x = np.asarray(x, np.float32)
    cos = np.asarray(cos, np.float32)
    sin = np.asarray(sin, np.float32)
    Wq = np.asarray(Wq, np.float32)
    Wk = np.asarray(Wk, np.float32)
    Wv = np.asarray(Wv, np.float32)
    Wo = np.asarray(Wo, np.float32)
    qn_w = np.asarray(qn_w, np.float32)
    kn_w = np.asarray(kn_w, np.float32)

    nc = _build()

    BF = ml_dtypes.bfloat16

    # xt: [NB, 128(d within ktile), DKT*128] blocks of x^T
    xt = np.ascontiguousarray(
        x.T.reshape(DKT, 128, NB, 128).transpose(2, 1, 0, 3)
    ).reshape(NB, 128, DKT * 128).astype(BF)

    # rope tables with rmsnorm sqrt(HD) and q/k norm weights folded in
    sgn = np.concatenate([-np.ones(HD // 2, np.float32),
                          np.ones(HD // 2, np.float32)])
    rt = float(np.sqrt(HD))
    cq = cos * (qn_w * rt)[None, :]
    sq = sin * (sgn * np.roll(qn_w, -(HD // 2)) * rt)[None, :]
    ck = cos * (kn_w * rt)[None, :]
    sk = sin * (sgn * np.roll(kn_w, -(HD // 2)) * rt)[None, :]

    tri_np = np.where(np.arange(128)[:, None] > np.arange(128)[None, :],
                      np.float32(0.0), np.float32(1.0))

    tblp = np.ascontiguousarray(
        np.stack([cq, sq, ck, sk]).astype(np.float32))
    base = dict(
        xt=xt, tbl=tblp,
        tri01=tri_np.astype(BF),
        ones128=np.ones((128, 128), BF),
        ident=np.eye(128, dtype=np.float32),
    )
    in_maps = []
    for cidx in range(NCORES):
        wq_c = Wq[:, cidx * HPC * HD:(cidx + 1) * HPC * HD]
        wq_t = np.ascontiguousarray(
            wq_c.reshape(DKT, 128, HPC * HD).transpose(1, 0, 2)
        ).reshape(128, DKT * HPC * HD).astype(BF)
        wk_c = Wk[:, cidx * HD:(cidx + 1) * HD]
        wv_c = Wv[:, cidx * HD:(cidx + 1) * HD]
        wkv_c = np.concatenate([wk_c, wv_c], axis=1)
        wkv_t = np.ascontiguousarray(
            wkv_c.reshape(DKT, 128, 256).transpose(1, 0, 2)
        ).reshape(128, DKT * 256).astype(BF)
        wo_c = Wo[cidx * HPC * HD:(cidx + 1) * HPC * HD, :]
        wo_t = np.ascontiguousarray(
            wo_c.reshape(HPC, HD, NOC, 512).transpose(1, 0, 2, 3)
        ).reshape(128, HPC * NOC * 512).astype(BF)
        in_maps.append(dict(base, wq=wq_t, wkv=wkv_t, wo=wo_t))

    res = run_bass_kernel_spmd(nc, in_maps, core_ids=list(range(NCORES)))
    acc = res.results[0]["out"].astype(np.float32)
    for r in res.results[1:]:
        acc = acc + r["out"].astype(np.float32)
    return acc
